# revision 22
# baseline (speedup 1.0000x reference)
"""H2GCNConv on 8 trn2 NeuronCores (Bass/Tile) — fused single-NEFF version.

Both mean-aggregation hops + final linear run in ONE SPMD program.
The axon tunnel to the device is slow (~0.13s fixed + ~60-90MB/s per
direction), so the design minimizes host<->device bytes and launches:

- x arrives dst-sharded (6250 rows/core) as per-row int8 with the f32
  row scale packed into 4 extra int8 columns (one 6.6MB staged array).
  Dequantized on DVE, then AllGathered (NeuronLink) into a full
  [50000,128] f32 gather source per core.
- Each hop: dma_gather chunks (lo/hi int16 source split) +
  dma_scatter_add into a per-hop 4-slot-expanded accumulator (slot
  expansion keeps indices unique per scatter — HBM scatter-add RMW
  races on duplicates; the scatter chain itself is serialized because
  different slot-groups may hit the same rows). Fold + 1/deg on DVE.
- hop1 shards are AllGathered on-device for hop2 (no host round-trip).
  All DMA is quiesced before each collective and accumulators are
  separate per hop — concurrent plain DMA or re-zeroing around a
  collective crashed the exec unit on HW (sim was clean).
- The linear (PE) output is emitted as per-row int8 + packed f32 scale
  (one 6.6MB fetched array); int8 convert truncates toward zero, so
  0.5*sign is added to round (halves the quantization error).
- The shard_map runner is AOT-compiled once with fast dispatch, and
  static index tensors stay device-resident, so a warm call pays only
  host quant (~35ms), one 6.6MB stage, ~40ms exec, one 6.6MB fetch.
"""
import sys
sys.path.insert(0, "/opt/trn_rl_repo")
import numpy as np
import concourse.bass as bass
import concourse.bacc as bacc
import concourse.tile as tile
mybir = bass.mybir
from concourse import bass2jax
from concourse.bass2jax import shard_map, Mesh, PartitionSpec

N, D, E, P = 50000, 128, 600000, 8
SH = N // P                      # 6250 rows per core
S = 32512                        # lo/hi split for int16 gather indices
NSLOT = 4
ARows = 6304
ACC_ROWS = NSLOT * ARows         # 25216
TRASH = 6272
CHUNK_MAX = 1024                 # largest dma_gather size verified crash-free
NT = 49                          # 49*128 = 6272 padded shard rows
SROWS = NT * 128

_CACHE = {}
_HOSTBUF = {}


def _wrap_idx(a):
    a = np.asarray(a, dtype=np.int16)
    n = a.shape[0]
    w = a.reshape(n // 16, 16).T.copy()
    return np.tile(w, (8, 1))


def _prep(edge_index):
    src = np.asarray(edge_index[0], dtype=np.int64)
    dst = np.asarray(edge_index[1], dtype=np.int64)
    deg = np.bincount(dst, minlength=N).astype(np.float32)
    inv_deg = (1.0 / np.maximum(deg, 1.0)).astype(np.float32)

    core_of = dst // SH
    order = np.argsort(dst, kind="stable")
    dsorted = dst[order]
    starts = np.searchsorted(dsorted, np.arange(N))
    rank_sorted = np.arange(E) - starts[dsorted]
    rank = np.empty(E, np.int64); rank[order] = rank_sorted
    sr = rank // NSLOT
    slot = rank % NSLOT
    half = (src >= S).astype(np.int64)
    n_sr = int(sr.max()) + 1

    key = core_of * (2 * n_sr) + sr * 2 + half
    ordk = np.argsort(key, kind="stable")
    ks = key[ordk]
    bounds = np.searchsorted(ks, np.arange(P * n_sr * 2 + 1))
    lists = [[[None, None] for _ in range(n_sr)] for _ in range(P)]
    for c in range(P):
        for t in range(n_sr):
            for h in (0, 1):
                k = c * (2 * n_sr) + t * 2 + h
                lists[c][t][h] = ordk[bounds[k]:bounds[k + 1]]

    sizes = [[max(len(lists[c][t][h]) for c in range(P)) for h in (0, 1)]
             for t in range(n_sr)]
    gidx = [[] for _ in range(P)]
    sidx = [[] for _ in range(P)]
    chunks = []
    for t in range(n_sr):
        for h in (0, 1):
            n_pad = -(-max(sizes[t][h], 1) // CHUNK_MAX) * CHUNK_MAX
            for c in range(P):
                el = lists[c][t][h]
                gs = src[el] - (S if h else 0)
                ss = (dst[el] - c * SH) + slot[el] * ARows
                npad = n_pad - len(el)
                gpad = np.zeros(npad, np.int64)        # pad gathers read row 0
                spad = TRASH + (np.arange(npad) % 24)  # pads land in trash rows
                gidx[c].append(np.concatenate([gs, gpad]))
                sidx[c].append(np.concatenate([ss, spad]))
            off = 0
            while off < n_pad:
                n = min(CHUNK_MAX, n_pad - off)
                chunks.append((h, n))
                off += n
    gidx = [np.concatenate(g) for g in gidx]
    sidx = [np.concatenate(s) for s in sidx]

    invc = []
    for c in range(P):
        v = np.zeros(NT * 128, np.float32)
        v[:SH] = inv_deg[c * SH:(c + 1) * SH]
        invc.append(v.reshape(NT, 128).T.copy())
    return dict(chunks=chunks, gidx=gidx, sidx=sidx, invc=invc)


def _build(chunks, total_idx):
    nc = bacc.Bacc(None, target_bir_lowering=False, debug=False, num_devices=P)
    dt = mybir.dt.float32
    i8 = mybir.dt.int8
    i16 = mybir.dt.int16
    CID = total_idx // 16
    GROUPS = [list(range(P))]

    # cols 0:128 = int8 row values, cols 128:132 = f32 row scale (bitcast)
    x_h = nc.dram_tensor("x_h", [SROWS, D + 4], i8, kind="ExternalInput")
    g_h = nc.dram_tensor("g_h", [128, CID], i16, kind="ExternalInput")
    s_h = nc.dram_tensor("s_h", [128, CID], i16, kind="ExternalInput")
    inv_h = nc.dram_tensor("inv_h", [128, NT], dt, kind="ExternalInput")
    wt_h = nc.dram_tensor("wt_h", [3 * D, D], dt, kind="ExternalInput")
    bias_h = nc.dram_tensor("bias_h", [128, D], dt, kind="ExternalInput")
    ident_h = nc.dram_tensor("ident_h", [128, 128], dt, kind="ExternalInput")
    out_h = nc.dram_tensor("out_h", [SROWS, D + 4], i8, kind="ExternalOutput")

    xs32 = nc.dram_tensor("xs32", [SROWS, D], dt)
    h1d = nc.dram_tensor("h1d", [SROWS, D], dt)
    T1 = nc.dram_tensor("T1", [N, D], dt, addr_space="Shared")
    T2 = nc.dram_tensor("T2", [N, D], dt, addr_space="Shared")
    acc1 = nc.dram_tensor("acc1", [ACC_ROWS, D], dt)
    acc2 = nc.dram_tensor("acc2", [ACC_ROWS, D], dt)

    def gate(*deps):
        n = None
        for d in deps:
            if d is None:
                continue
            if n is None:
                n = nc.gpsimd.nop()
            bass._add_dep_helper(n.ins, d.ins, sync=True, reason="gate")
        return n

    with tile.TileContext(nc) as tc:
        with tc.tile_pool(name="pc", bufs=1) as pc, \
             tc.tile_pool(name="gp", bufs=3) as gp, \
             tc.tile_pool(name="hp", bufs=3) as hp, \
             tc.tile_pool(name="pp", bufs=4, space="PSUM") as pp:
            # static loads
            gix = pc.tile([128, CID], i16)
            six = pc.tile([128, CID], i16)
            dg1 = nc.sync.dma_start(out=gix[:], in_=g_h[:])
            dg2 = nc.sync.dma_start(out=six[:], in_=s_h[:])
            inv_t = pc.tile([128, NT], dt)
            nc.sync.dma_start(out=inv_t[:], in_=inv_h[:])
            ident = pc.tile([128, 128], dt)
            nc.sync.dma_start(out=ident[:], in_=ident_h[:])
            wt_t = pc.tile([128, 3, D], dt)
            nc.sync.dma_start(out=wt_t[:], in_=wt_h[:].rearrange("(k p) d -> p k d", p=128))
            bias_t = pc.tile([128, D], dt)
            nc.sync.dma_start(out=bias_t[:], in_=bias_h[:])

            # x int8 + packed f32 row scale -> f32, keep tiles in SBUF,
            # write xs32 for AllGather
            xq = pc.tile([128, NT, D], i8)
            nc.sync.dma_start(
                out=xq[:], in_=x_h[:, 0:D].rearrange("(t p) d -> p t d", p=128))
            xsc4 = pc.tile([128, NT, 4], i8)
            nc.sync.dma_start(
                out=xsc4[:], in_=x_h[:, D:D + 4].rearrange("(t p) c -> p t c", p=128))
            xsc = xsc4[:].bitcast(dt)
            xconv = pc.tile([128, NT, D], dt)
            nc.vector.tensor_copy(xconv[:], xq[:])
            for t in range(NT):
                nc.vector.tensor_scalar_mul(
                    xconv[:, t, :], xconv[:, t, :], xsc[:, t, 0:1])
            dxs = nc.sync.dma_start(
                out=xs32[:].rearrange("(t p) d -> p t d", p=128), in_=xconv[:])

            # zero both accumulators up front
            zt = pc.tile([128, 2048], dt)
            nc.vector.memset(zt[:], 0.0)
            zds = []
            total = ACC_ROWS * D // 128
            for a in (acc1, acc2):
                flat = a[:].rearrange("r d -> (r d)").rearrange("(p f) -> p f", p=128)
                o = 0
                while o < total:
                    n = min(2048, total - o)
                    zds.append(nc.sync.dma_start(out=flat[:, o:o + n], in_=zt[:, :n]))
                    o += n

            # AllGather x shards -> full T1 (quiesce all DMA first)
            gate(dxs, dg1, dg2, *zds)
            cc1 = nc.gpsimd.collective_compute(
                "AllGather", mybir.AluOpType.bypass, replica_groups=GROUPS,
                ins=[xs32[0:SH, :].opt()], outs=[T1[:].opt()])

            def hop(src_t, acc, first_gates, tagp):
                off = 0
                last_sc = None
                first = True
                for (h, n) in chunks:
                    assert n == CHUNK_MAX
                    gt = gp.tile([128, CHUNK_MAX // 128, D], dt, tag=f"gt{tagp}")
                    cgi = gp.tile([128, CHUNK_MAX // 16], i16, tag=f"cgi{tagp}")
                    csi = gp.tile([128, CHUNK_MAX // 16], i16, tag=f"csi{tagp}")
                    c1 = nc.vector.tensor_copy(cgi[:], gix[:, off:off + n // 16])
                    c2 = nc.vector.tensor_copy(csi[:], six[:, off:off + n // 16])
                    gate(c1)
                    if first:
                        gate(*first_gates)
                        first = False
                    g = nc.gpsimd.dma_gather(
                        gt[:],
                        src_t[S:N, :] if h else src_t[0:S, :],
                        cgi[:], n, n, D)
                    # gathers run ahead; only the scatter chain serializes
                    # (scatters may hit the same acc rows -> HBM RMW races)
                    gate(g, c2, last_sc)
                    last_sc = nc.gpsimd.dma_scatter_add(
                        acc[:], gt[:], csi[:], n, n, D)
                    off += n // 16
                return last_sc

            def fold(acc, tagp, write_hbm):
                tiles, lds, whs = [], [], []
                accv = acc[:].rearrange("(s r) d -> s r d", s=NSLOT)
                for t in range(NT):
                    ft = hp.tile([128, NSLOT, D], dt, tag=f"fold{tagp}")
                    ld = nc.sync.dma_start(
                        out=ft[:],
                        in_=accv[:, t * 128:(t + 1) * 128, :].rearrange("s r d -> r s d"))
                    ht = pc.tile([128, D], dt, tag=f"h{tagp}_{t}")
                    nc.vector.tensor_tensor(out=ht[:], in0=ft[:, 0, :], in1=ft[:, 1, :],
                                            op=mybir.AluOpType.add)
                    nc.vector.tensor_tensor(out=ht[:], in0=ht[:], in1=ft[:, 2, :],
                                            op=mybir.AluOpType.add)
                    nc.vector.tensor_tensor(out=ht[:], in0=ht[:], in1=ft[:, 3, :],
                                            op=mybir.AluOpType.add)
                    nc.vector.tensor_scalar_mul(ht[:], ht[:], inv_t[:, t:t + 1])
                    if write_hbm:
                        wh = nc.sync.dma_start(
                            out=h1d[t * 128:(t + 1) * 128, :], in_=ht[:])
                        whs.append(wh)
                    tiles.append(ht)
                    lds.append(ld)
                return tiles, lds, whs

            # hop 1: x -> h1
            sc1 = hop(T1, acc1, [cc1], "a")
            gate(sc1)
            h1_tiles, lds1, whs1 = fold(acc1, "a", True)

            # AllGather h1 shards -> full T2 (quiesce fold DMA first)
            gate(*whs1, *lds1)
            cc2 = nc.gpsimd.collective_compute(
                "AllGather", mybir.AluOpType.bypass, replica_groups=GROUPS,
                ins=[h1d[0:SH, :].opt()], outs=[T2[:].opt()])

            # hop 2: h1 -> h2 (tiles only, no HBM write)
            sc2 = hop(T2, acc2, [cc2], "b")
            gate(sc2)
            h2_tiles, lds2, _ = fold(acc2, "b", False)

            # linear: out = [x | h1 | h2] @ W.T + b, emitted fp16
            for t in range(NT):
                po = pp.tile([128, D], dt, tag="po")
                feats = [xconv[:, t, :], h1_tiles[t][:], h2_tiles[t][:]]
                for j, fap in enumerate(feats):
                    pt = pp.tile([128, D], dt, tag="pt")
                    nc.tensor.transpose(pt[:], fap, ident[:])
                    st = hp.tile([128, D], dt, tag="st")
                    nc.vector.tensor_copy(st[:], pt[:])
                    nc.tensor.matmul(po[:], st[:], wt_t[:, j, :],
                                     start=(j == 0), stop=(j == 2))
                ot = hp.tile([128, D], dt, tag="ot")
                nc.vector.tensor_tensor(out=ot[:], in0=po[:], in1=bias_t[:],
                                        op=mybir.AluOpType.add)
                rmax = hp.tile([128, 1], dt, tag="rmax")
                nc.vector.tensor_reduce(rmax[:], ot[:], mybir.AxisListType.X,
                                        mybir.AluOpType.max,
                                        apply_absolute_value=True)
                nc.vector.tensor_scalar_max(rmax[:], rmax[:], 1e-30)
                rinv = hp.tile([128, 1], dt, tag="rinv")
                nc.vector.reciprocal(rinv[:], rmax[:])
                qf = hp.tile([128, D], dt, tag="qf")
                nc.vector.tensor_scalar(out=qf[:], in0=ot[:],
                                        scalar1=rinv[:, 0:1], scalar2=126.0,
                                        op0=mybir.AluOpType.mult,
                                        op1=mybir.AluOpType.mult)
                # int8 convert truncates toward zero: add 0.5*sign to round
                sg = hp.tile([128, D], dt, tag="sg")
                nc.scalar.activation(sg[:], qf[:],
                                     mybir.ActivationFunctionType.Sign)
                nc.vector.tensor_scalar_mul(sg[:], sg[:], 0.5)
                q8 = hp.tile([128, D], i8, tag="q8")
                nc.vector.tensor_tensor(out=q8[:], in0=qf[:], in1=sg[:],
                                        op=mybir.AluOpType.add)
                nc.sync.dma_start(out=out_h[t * 128:(t + 1) * 128, 0:D], in_=q8[:])
                nc.sync.dma_start(out=out_h[t * 128:(t + 1) * 128, D:D + 4],
                                  in_=rmax[:].bitcast(i8))

    nc.finalize()
    return nc


def _make_runner(nc):
    import jax
    from jax.sharding import NamedSharding

    bass2jax.install_neuronx_cc_hook()
    partition_name = nc.partition_id_tensor.name if nc.partition_id_tensor else None

    in_names = []
    out_names = []
    out_avals = []
    for alloc in nc.m.functions[0].allocations:
        if not isinstance(alloc, mybir.MemoryLocationSet):
            continue
        name = alloc.memorylocations[0].name if alloc.memorylocations else None
        if alloc.kind == "ExternalInput":
            if name != partition_name:
                in_names.append(name)
        elif alloc.kind == "ExternalOutput":
            out_names.append(name)
            out_avals.append(jax.core.ShapedArray(
                tuple(alloc.tensor_shape), mybir.dt.np(alloc.dtype)))

    devices = jax.devices()[:P]
    mesh = Mesh(np.asarray(devices), ("core",))
    bind_in_names = tuple(in_names + ([partition_name] if partition_name else []))

    def _body(*args):
        operands = list(args)
        if partition_name is not None:
            operands.append(bass2jax.partition_id_tensor())
        outs = bass2jax._bass_exec_p.bind(
            *operands,
            out_avals=tuple(out_avals),
            in_names=bind_in_names,
            out_names=tuple(out_names),
            lowering_input_output_aliases=(),
            sim_require_finite=True,
            sim_require_nnan=True,
            nc=nc,
        )
        return tuple(outs)

    sharded = shard_map(
        _body, mesh=mesh,
        in_specs=(PartitionSpec("core"),) * len(in_names),
        out_specs=(PartitionSpec("core"),) * len(out_names),
        check_rep=False)
    sharding = NamedSharding(mesh, PartitionSpec("core"))

    avals = []
    for alloc in nc.m.functions[0].allocations:
        if not isinstance(alloc, mybir.MemoryLocationSet):
            continue
        name = alloc.memorylocations[0].name if alloc.memorylocations else None
        if alloc.kind == "ExternalInput" and name != partition_name:
            shape = tuple(alloc.tensor_shape)
            avals.append(jax.ShapeDtypeStruct(
                (shape[0] * P, *shape[1:]), mybir.dt.np(alloc.dtype),
                sharding=sharding))
    try:
        fn = bass2jax.fast_dispatch_compile(
            lambda: jax.jit(sharded).lower(*avals).compile())
    except Exception:
        fn = jax.jit(sharded)
    return fn, in_names, out_names, sharding


def kernel(x, edge_index, W, b):
    import jax

    x = np.asarray(x, np.float32)
    W = np.asarray(W, np.float32)
    b = np.asarray(b, np.float32)
    ekey = hash(np.asarray(edge_index).tobytes())
    if ekey not in _CACHE:
        pre = _prep(edge_index)
        total_idx = len(pre["gidx"][0])
        nc = _build(pre["chunks"], total_idx)
        fn, in_names, out_names, sharding = _make_runner(nc)

        ident = np.eye(128, dtype=np.float32)
        statics = {
            "g_h": np.concatenate([_wrap_idx(g) for g in pre["gidx"]], axis=0),
            "s_h": np.concatenate([_wrap_idx(s) for s in pre["sidx"]], axis=0),
            "inv_h": np.concatenate(pre["invc"], axis=0),
            "wt_h": np.concatenate([np.ascontiguousarray(W.T)] * P, axis=0),
            "bias_h": np.concatenate(
                [np.tile(b[None, :], (128, 1))] * P, axis=0).astype(np.float32),
            "ident_h": np.concatenate([ident] * P, axis=0),
        }
        static_dev = {k: jax.device_put(v, sharding) for k, v in statics.items()}
        for v in static_dev.values():
            v.block_until_ready()
        _CACHE.clear()
        _CACHE[ekey] = (fn, in_names, out_names, static_dev)
    fn, in_names, out_names, static_dev = _CACHE[ekey]

    # per-row int8 quantization of x, scale packed as f32 in cols 128:132
    if "xp" not in _HOSTBUF:
        _HOSTBUF["xp"] = np.zeros((P, SROWS, D + 4), np.int8)
        _HOSTBUF["y"] = np.empty((N, D), np.float32)
    xp, y = _HOSTBUF["xp"], _HOSTBUF["y"]
    m = np.abs(x, out=y).max(axis=1)
    factor = np.where(m > 0, 126.0 / m, 0.0).astype(np.float32)
    scale = (m / 126.0).astype(np.float32)
    np.multiply(x, factor[:, None], out=y)
    np.rint(y, out=y)
    xp[:, :SH, 0:D] = y.astype(np.int8).reshape(P, SH, D)
    xp[:, :SH, D:D + 4] = scale.view(np.int8).reshape(P, SH, 4)
    xpf = xp.reshape(P * SROWS, D + 4)

    args = []
    for name in in_names:
        if name == "x_h":
            args.append(xpf)
        else:
            args.append(static_dev[name])
    outs = fn(*args)
    o = outs[out_names.index("out_h")]
    out = np.empty((N, D), np.float32)

    def _dequant(shard):
        c = shard.index[0].start // SROWS
        buf = np.asarray(shard.data)[:SH]
        sc = np.ascontiguousarray(buf[:, D:D + 4]).view(np.float32)
        np.multiply(sc, np.float32(1.0 / 126.0), out=sc)
        np.multiply(buf[:, 0:D], sc, out=out[c * SH:(c + 1) * SH])

    from concurrent.futures import ThreadPoolExecutor
    with ThreadPoolExecutor(P) as ex:
        list(ex.map(_dequant, o.addressable_shards))
    return out


# revision 23
# speedup vs baseline: 1.2330x; 1.2330x over previous
"""H2GCNConv on 8 trn2 NeuronCores (Bass/Tile) — fused single-NEFF version.

Both mean-aggregation hops + final linear run in ONE SPMD program.
The axon tunnel to the device is slow (~0.13s fixed + ~60-90MB/s per
direction), so the design minimizes host<->device bytes and launches:

- x arrives dst-sharded (6250 rows/core) as per-row int8 with the f32
  row scale packed into 4 extra int8 columns (one 6.6MB staged array).
  Dequantized on DVE, then AllGathered (NeuronLink) into a full
  [50000,128] f32 gather source per core.
- Each hop: dma_gather chunks (lo/hi int16 source split) +
  dma_scatter_add into a per-hop 4-slot-expanded accumulator (slot
  expansion keeps indices unique per scatter — HBM scatter-add RMW
  races on duplicates; the scatter chain itself is serialized because
  different slot-groups may hit the same rows). Fold + 1/deg on DVE.
- hop1 shards are AllGathered on-device for hop2 (no host round-trip).
  All DMA is quiesced before each collective and accumulators are
  separate per hop — concurrent plain DMA or re-zeroing around a
  collective crashed the exec unit on HW (sim was clean).
- The linear (PE) output is emitted as per-row int8 + packed f32 scale
  (one 6.6MB fetched array); int8 convert truncates toward zero, so
  0.5*sign is added to round (halves the quantization error).
- The shard_map runner is AOT-compiled once with fast dispatch, and
  static index tensors stay device-resident, so a warm call pays only
  host quant (~35ms), one 6.6MB stage, ~40ms exec, one 6.6MB fetch.
"""
import sys
sys.path.insert(0, "/opt/trn_rl_repo")
import numpy as np
import concourse.bass as bass
import concourse.bacc as bacc
import concourse.tile as tile
mybir = bass.mybir
from concourse import bass2jax
from concourse.bass2jax import shard_map, Mesh, PartitionSpec

N, D, E, P = 50000, 128, 600000, 8
SH = N // P                      # 6250 rows per core
S = 32512                        # lo/hi split for int16 gather indices
NSLOT = 4
ARows = 6304
ACC_ROWS = NSLOT * ARows         # 25216
TRASH = 6272
CHUNK_MAX = 1024                 # largest dma_gather size verified crash-free
NT = 49                          # 49*128 = 6272 padded shard rows
SROWS = NT * 128

_CACHE = {}
_HOSTBUF = {}


def _wrap_idx(a):
    a = np.asarray(a, dtype=np.int16)
    n = a.shape[0]
    w = a.reshape(n // 16, 16).T.copy()
    return np.tile(w, (8, 1))


def _prep(edge_index):
    src = np.asarray(edge_index[0], dtype=np.int64)
    dst = np.asarray(edge_index[1], dtype=np.int64)
    deg = np.bincount(dst, minlength=N).astype(np.float32)
    inv_deg = (1.0 / np.maximum(deg, 1.0)).astype(np.float32)

    core_of = dst // SH
    order = np.argsort(dst, kind="stable")
    dsorted = dst[order]
    starts = np.searchsorted(dsorted, np.arange(N))
    rank_sorted = np.arange(E) - starts[dsorted]
    rank = np.empty(E, np.int64); rank[order] = rank_sorted
    sr = rank // NSLOT
    slot = rank % NSLOT
    half = (src >= S).astype(np.int64)
    n_sr = int(sr.max()) + 1

    key = core_of * (2 * n_sr) + sr * 2 + half
    ordk = np.argsort(key, kind="stable")
    ks = key[ordk]
    bounds = np.searchsorted(ks, np.arange(P * n_sr * 2 + 1))
    lists = [[[None, None] for _ in range(n_sr)] for _ in range(P)]
    for c in range(P):
        for t in range(n_sr):
            for h in (0, 1):
                k = c * (2 * n_sr) + t * 2 + h
                lists[c][t][h] = ordk[bounds[k]:bounds[k + 1]]

    sizes = [[max(len(lists[c][t][h]) for c in range(P)) for h in (0, 1)]
             for t in range(n_sr)]
    gidx = [[] for _ in range(P)]
    sidx = [[] for _ in range(P)]
    chunks = []
    for t in range(n_sr):
        for h in (0, 1):
            n_pad = -(-max(sizes[t][h], 1) // CHUNK_MAX) * CHUNK_MAX
            for c in range(P):
                el = lists[c][t][h]
                gs = src[el] - (S if h else 0)
                ss = (dst[el] - c * SH) + slot[el] * ARows
                npad = n_pad - len(el)
                gpad = np.zeros(npad, np.int64)        # pad gathers read row 0
                spad = TRASH + (np.arange(npad) % 24)  # pads land in trash rows
                gidx[c].append(np.concatenate([gs, gpad]))
                sidx[c].append(np.concatenate([ss, spad]))
            off = 0
            while off < n_pad:
                n = min(CHUNK_MAX, n_pad - off)
                chunks.append((h, n))
                off += n
    gidx = [np.concatenate(g) for g in gidx]
    sidx = [np.concatenate(s) for s in sidx]

    invc = []
    for c in range(P):
        v = np.zeros(NT * 128, np.float32)
        v[:SH] = inv_deg[c * SH:(c + 1) * SH]
        invc.append(v.reshape(NT, 128).T.copy())
    return dict(chunks=chunks, gidx=gidx, sidx=sidx, invc=invc)


def _build(chunks, total_idx):
    nc = bacc.Bacc(None, target_bir_lowering=False, debug=False, num_devices=P)
    dt = mybir.dt.float32
    i8 = mybir.dt.int8
    i16 = mybir.dt.int16
    CID = total_idx // 16
    GROUPS = [list(range(P))]

    # cols 0:128 = int8 row values, cols 128:132 = f32 row scale (bitcast)
    x_h = nc.dram_tensor("x_h", [SROWS, D + 4], i8, kind="ExternalInput")
    g_h = nc.dram_tensor("g_h", [128, CID], i16, kind="ExternalInput")
    s_h = nc.dram_tensor("s_h", [128, CID], i16, kind="ExternalInput")
    inv_h = nc.dram_tensor("inv_h", [128, NT], dt, kind="ExternalInput")
    wt_h = nc.dram_tensor("wt_h", [3 * D, D], dt, kind="ExternalInput")
    bias_h = nc.dram_tensor("bias_h", [128, D], dt, kind="ExternalInput")
    ident_h = nc.dram_tensor("ident_h", [128, 128], dt, kind="ExternalInput")
    out_h = nc.dram_tensor("out_h", [SROWS, D + 4], i8, kind="ExternalOutput")

    xs32 = nc.dram_tensor("xs32", [SROWS, D], dt)
    h1d = nc.dram_tensor("h1d", [SROWS, D], dt)
    T1 = nc.dram_tensor("T1", [N, D], dt, addr_space="Shared")
    T2 = nc.dram_tensor("T2", [N, D], dt, addr_space="Shared")
    acc1 = nc.dram_tensor("acc1", [ACC_ROWS, D], dt)
    acc2 = nc.dram_tensor("acc2", [ACC_ROWS, D], dt)

    def gate(*deps):
        n = None
        for d in deps:
            if d is None:
                continue
            if n is None:
                n = nc.gpsimd.nop()
            bass._add_dep_helper(n.ins, d.ins, sync=True, reason="gate")
        return n

    with tile.TileContext(nc) as tc:
        with tc.tile_pool(name="pc", bufs=1) as pc, \
             tc.tile_pool(name="gp", bufs=3) as gp, \
             tc.tile_pool(name="hp", bufs=3) as hp, \
             tc.tile_pool(name="pp", bufs=4, space="PSUM") as pp:
            # static loads
            gix = pc.tile([128, CID], i16)
            six = pc.tile([128, CID], i16)
            dg1 = nc.sync.dma_start(out=gix[:], in_=g_h[:])
            dg2 = nc.sync.dma_start(out=six[:], in_=s_h[:])
            inv_t = pc.tile([128, NT], dt)
            nc.sync.dma_start(out=inv_t[:], in_=inv_h[:])
            ident = pc.tile([128, 128], dt)
            nc.sync.dma_start(out=ident[:], in_=ident_h[:])
            wt_t = pc.tile([128, 3, D], dt)
            nc.sync.dma_start(out=wt_t[:], in_=wt_h[:].rearrange("(k p) d -> p k d", p=128))
            bias_t = pc.tile([128, D], dt)
            nc.sync.dma_start(out=bias_t[:], in_=bias_h[:])

            # x int8 + packed f32 row scale -> f32, keep tiles in SBUF,
            # write xs32 for AllGather
            xq = pc.tile([128, NT, D], i8)
            nc.sync.dma_start(
                out=xq[:], in_=x_h[:, 0:D].rearrange("(t p) d -> p t d", p=128))
            xsc4 = pc.tile([128, NT, 4], i8)
            nc.sync.dma_start(
                out=xsc4[:], in_=x_h[:, D:D + 4].rearrange("(t p) c -> p t c", p=128))
            xsc = xsc4[:].bitcast(dt)
            xconv = pc.tile([128, NT, D], dt)
            nc.vector.tensor_copy(xconv[:], xq[:])
            for t in range(NT):
                nc.vector.tensor_scalar_mul(
                    xconv[:, t, :], xconv[:, t, :], xsc[:, t, 0:1])
            dxs = nc.sync.dma_start(
                out=xs32[:].rearrange("(t p) d -> p t d", p=128), in_=xconv[:])

            # zero both accumulators up front
            zt = pc.tile([128, 2048], dt)
            nc.vector.memset(zt[:], 0.0)
            zds = []
            total = ACC_ROWS * D // 128
            for a in (acc1, acc2):
                flat = a[:].rearrange("r d -> (r d)").rearrange("(p f) -> p f", p=128)
                o = 0
                while o < total:
                    n = min(2048, total - o)
                    zds.append(nc.sync.dma_start(out=flat[:, o:o + n], in_=zt[:, :n]))
                    o += n

            # AllGather x shards -> full T1 (quiesce all DMA first)
            gate(dxs, dg1, dg2, *zds)
            cc1 = nc.gpsimd.collective_compute(
                "AllGather", mybir.AluOpType.bypass, replica_groups=GROUPS,
                ins=[xs32[0:SH, :].opt()], outs=[T1[:].opt()])

            def hop(src_t, acc, first_gates, tagp):
                off = 0
                last_sc = None
                first = True
                for (h, n) in chunks:
                    assert n == CHUNK_MAX
                    gt = gp.tile([128, CHUNK_MAX // 128, D], dt, tag=f"gt{tagp}")
                    cgi = gp.tile([128, CHUNK_MAX // 16], i16, tag=f"cgi{tagp}")
                    csi = gp.tile([128, CHUNK_MAX // 16], i16, tag=f"csi{tagp}")
                    c1 = nc.vector.tensor_copy(cgi[:], gix[:, off:off + n // 16])
                    c2 = nc.vector.tensor_copy(csi[:], six[:, off:off + n // 16])
                    gate(c1)
                    if first:
                        gate(*first_gates)
                        first = False
                    g = nc.gpsimd.dma_gather(
                        gt[:],
                        src_t[S:N, :] if h else src_t[0:S, :],
                        cgi[:], n, n, D)
                    # gathers run ahead; only the scatter chain serializes
                    # (scatters may hit the same acc rows -> HBM RMW races)
                    gate(g, c2, last_sc)
                    last_sc = nc.gpsimd.dma_scatter_add(
                        acc[:], gt[:], csi[:], n, n, D)
                    off += n // 16
                return last_sc

            def fold(acc, tagp, write_hbm):
                tiles, lds, whs = [], [], []
                accv = acc[:].rearrange("(s r) d -> s r d", s=NSLOT)
                for t in range(NT):
                    ft = hp.tile([128, NSLOT, D], dt, tag=f"fold{tagp}")
                    ld = nc.sync.dma_start(
                        out=ft[:],
                        in_=accv[:, t * 128:(t + 1) * 128, :].rearrange("s r d -> r s d"))
                    ht = pc.tile([128, D], dt, tag=f"h{tagp}_{t}")
                    nc.vector.tensor_tensor(out=ht[:], in0=ft[:, 0, :], in1=ft[:, 1, :],
                                            op=mybir.AluOpType.add)
                    nc.vector.tensor_tensor(out=ht[:], in0=ht[:], in1=ft[:, 2, :],
                                            op=mybir.AluOpType.add)
                    nc.vector.tensor_tensor(out=ht[:], in0=ht[:], in1=ft[:, 3, :],
                                            op=mybir.AluOpType.add)
                    nc.vector.tensor_scalar_mul(ht[:], ht[:], inv_t[:, t:t + 1])
                    if write_hbm:
                        wh = nc.sync.dma_start(
                            out=h1d[t * 128:(t + 1) * 128, :], in_=ht[:])
                        whs.append(wh)
                    tiles.append(ht)
                    lds.append(ld)
                return tiles, lds, whs

            # hop 1: x -> h1
            sc1 = hop(T1, acc1, [cc1], "a")
            gate(sc1)
            h1_tiles, lds1, whs1 = fold(acc1, "a", True)

            # AllGather h1 shards -> full T2 (quiesce fold DMA first)
            gate(*whs1, *lds1)
            cc2 = nc.gpsimd.collective_compute(
                "AllGather", mybir.AluOpType.bypass, replica_groups=GROUPS,
                ins=[h1d[0:SH, :].opt()], outs=[T2[:].opt()])

            # hop 2: h1 -> h2 (tiles only, no HBM write)
            sc2 = hop(T2, acc2, [cc2], "b")
            gate(sc2)
            h2_tiles, lds2, _ = fold(acc2, "b", False)

            # linear: out = [x | h1 | h2] @ W.T + b, emitted fp16
            for t in range(NT):
                po = pp.tile([128, D], dt, tag="po")
                feats = [xconv[:, t, :], h1_tiles[t][:], h2_tiles[t][:]]
                for j, fap in enumerate(feats):
                    pt = pp.tile([128, D], dt, tag="pt")
                    nc.tensor.transpose(pt[:], fap, ident[:])
                    st = hp.tile([128, D], dt, tag="st")
                    nc.vector.tensor_copy(st[:], pt[:])
                    nc.tensor.matmul(po[:], st[:], wt_t[:, j, :],
                                     start=(j == 0), stop=(j == 2))
                ot = hp.tile([128, D], dt, tag="ot")
                nc.vector.tensor_tensor(out=ot[:], in0=po[:], in1=bias_t[:],
                                        op=mybir.AluOpType.add)
                rmax = hp.tile([128, 1], dt, tag="rmax")
                nc.vector.tensor_reduce(rmax[:], ot[:], mybir.AxisListType.X,
                                        mybir.AluOpType.max,
                                        apply_absolute_value=True)
                nc.vector.tensor_scalar_max(rmax[:], rmax[:], 1e-30)
                rinv = hp.tile([128, 1], dt, tag="rinv")
                nc.vector.reciprocal(rinv[:], rmax[:])
                qf = hp.tile([128, D], dt, tag="qf")
                nc.vector.tensor_scalar(out=qf[:], in0=ot[:],
                                        scalar1=rinv[:, 0:1], scalar2=126.0,
                                        op0=mybir.AluOpType.mult,
                                        op1=mybir.AluOpType.mult)
                # int8 convert truncates toward zero: add 0.5*sign to round
                sg = hp.tile([128, D], dt, tag="sg")
                nc.scalar.activation(sg[:], qf[:],
                                     mybir.ActivationFunctionType.Sign)
                nc.vector.tensor_scalar_mul(sg[:], sg[:], 0.5)
                q8 = hp.tile([128, D], i8, tag="q8")
                nc.vector.tensor_tensor(out=q8[:], in0=qf[:], in1=sg[:],
                                        op=mybir.AluOpType.add)
                nc.sync.dma_start(out=out_h[t * 128:(t + 1) * 128, 0:D], in_=q8[:])
                nc.sync.dma_start(out=out_h[t * 128:(t + 1) * 128, D:D + 4],
                                  in_=rmax[:].bitcast(i8))

    nc.finalize()
    return nc


def _make_runner(nc):
    import jax
    from jax.sharding import NamedSharding

    bass2jax.install_neuronx_cc_hook()
    partition_name = nc.partition_id_tensor.name if nc.partition_id_tensor else None

    in_names = []
    out_names = []
    out_avals = []
    for alloc in nc.m.functions[0].allocations:
        if not isinstance(alloc, mybir.MemoryLocationSet):
            continue
        name = alloc.memorylocations[0].name if alloc.memorylocations else None
        if alloc.kind == "ExternalInput":
            if name != partition_name:
                in_names.append(name)
        elif alloc.kind == "ExternalOutput":
            out_names.append(name)
            out_avals.append(jax.core.ShapedArray(
                tuple(alloc.tensor_shape), mybir.dt.np(alloc.dtype)))

    devices = jax.devices()[:P]
    mesh = Mesh(np.asarray(devices), ("core",))
    bind_in_names = tuple(in_names + ([partition_name] if partition_name else []))

    def _body(*args):
        operands = list(args)
        if partition_name is not None:
            operands.append(bass2jax.partition_id_tensor())
        outs = bass2jax._bass_exec_p.bind(
            *operands,
            out_avals=tuple(out_avals),
            in_names=bind_in_names,
            out_names=tuple(out_names),
            lowering_input_output_aliases=(),
            sim_require_finite=True,
            sim_require_nnan=True,
            nc=nc,
        )
        return tuple(outs)

    sharded = shard_map(
        _body, mesh=mesh,
        in_specs=(PartitionSpec("core"),) * len(in_names),
        out_specs=(PartitionSpec("core"),) * len(out_names),
        check_rep=False)
    sharding = NamedSharding(mesh, PartitionSpec("core"))

    avals = []
    for alloc in nc.m.functions[0].allocations:
        if not isinstance(alloc, mybir.MemoryLocationSet):
            continue
        name = alloc.memorylocations[0].name if alloc.memorylocations else None
        if alloc.kind == "ExternalInput" and name != partition_name:
            shape = tuple(alloc.tensor_shape)
            avals.append(jax.ShapeDtypeStruct(
                (shape[0] * P, *shape[1:]), mybir.dt.np(alloc.dtype),
                sharding=sharding))
    try:
        fn = bass2jax.fast_dispatch_compile(
            lambda: jax.jit(sharded).lower(*avals).compile())
    except Exception:
        fn = jax.jit(sharded)
    return fn, in_names, out_names, sharding


def kernel(x, edge_index, W, b):
    import jax

    x = np.asarray(x, np.float32)
    W = np.asarray(W, np.float32)
    b = np.asarray(b, np.float32)
    ekey = hash(np.asarray(edge_index).tobytes())
    if ekey not in _CACHE:
        pre = _prep(edge_index)
        total_idx = len(pre["gidx"][0])
        nc = _build(pre["chunks"], total_idx)
        fn, in_names, out_names, sharding = _make_runner(nc)

        ident = np.eye(128, dtype=np.float32)
        statics = {
            "g_h": np.concatenate([_wrap_idx(g) for g in pre["gidx"]], axis=0),
            "s_h": np.concatenate([_wrap_idx(s) for s in pre["sidx"]], axis=0),
            "inv_h": np.concatenate(pre["invc"], axis=0),
            "wt_h": np.concatenate([np.ascontiguousarray(W.T)] * P, axis=0),
            "bias_h": np.concatenate(
                [np.tile(b[None, :], (128, 1))] * P, axis=0).astype(np.float32),
            "ident_h": np.concatenate([ident] * P, axis=0),
        }
        static_dev = {k: jax.device_put(v, sharding) for k, v in statics.items()}
        for v in static_dev.values():
            v.block_until_ready()
        _CACHE.clear()
        _CACHE[ekey] = (fn, in_names, out_names, static_dev)
    fn, in_names, out_names, static_dev = _CACHE[ekey]

    # per-row int8 quantization of x, scale packed as f32 in cols 128:132
    if "xp" not in _HOSTBUF:
        _HOSTBUF["xp"] = np.zeros((P, SROWS, D + 4), np.int8)
    xp = _HOSTBUF["xp"]

    def _quant(c):
        xs = x[c * SH:(c + 1) * SH]
        m = np.abs(xs).max(axis=1)
        factor = np.where(m > 0, 126.0 / m, 0.0).astype(np.float32)
        scale = (m / 126.0).astype(np.float32)
        y = xs * factor[:, None]
        np.rint(y, out=y)
        xp[c, :SH, 0:D] = y.astype(np.int8)
        xp[c, :SH, D:D + 4] = scale.view(np.int8).reshape(SH, 4)

    from concurrent.futures import ThreadPoolExecutor
    with ThreadPoolExecutor(P) as ex:
        list(ex.map(_quant, range(P)))
    xpf = xp.reshape(P * SROWS, D + 4)

    args = []
    for name in in_names:
        if name == "x_h":
            args.append(xpf)
        else:
            args.append(static_dev[name])
    outs = fn(*args)
    o = outs[out_names.index("out_h")]
    out = np.empty((N, D), np.float32)

    def _dequant(shard):
        c = shard.index[0].start // SROWS
        buf = np.asarray(shard.data)[:SH]
        sc = np.ascontiguousarray(buf[:, D:D + 4]).view(np.float32)
        np.multiply(sc, np.float32(1.0 / 126.0), out=sc)
        np.multiply(buf[:, 0:D], sc, out=out[c * SH:(c + 1) * SH])

    from concurrent.futures import ThreadPoolExecutor
    with ThreadPoolExecutor(P) as ex:
        list(ex.map(_dequant, o.addressable_shards))
    return out
